# revision 6
# baseline (speedup 1.0000x reference)
"""Bahdanau attention forward on 8 Trainium2 NeuronCores (Bass/Tile).

Problem (per reference):
    query = hidden[-1]                                   [B, D]
    proj  = enc @ W1.T + W1_b + query @ W2.T + W2_b      [B, S, H]
    score = tanh(proj) @ V_w[0] + V_b                    [B, S]
    score = where(mask == 0, -1e9, score)
    attn  = softmax(score, axis=1)                       [B, S]
    ctx   = attn @ enc                                   [B, D]
    returns (ctx, attn)

Sharding: data-parallel over batch. B=32 -> 4 batches per core; params
replicated. Each core streams its 4x[S=2048, D=1024] encoder slice from HBM
exactly once (memory roofline ~32MB/core).

Per-core pipeline, per 512-row s-chunk:
    SWDGE cast-load enc f32->bf16 (natural [s,d])
    PE transpose 128x128 blocks -> encT [d,s] bf16 (proj needs d on partitions)
    PE proj matmuls (W1T stationary)  -> PSUM [h=128x4, s=512] f32
    ACT tanh(proj + qp_b) -> bf16     (qp = W2@q + b1 + b2, per-partition bias)
    PE score matmuls (V stationary)   -> PSUM [1, 512] f32
    ACT exp(score + V_b)              (unnormalized softmax; exp and tanh share
                                       one ACT table set)
    DVE exp*mask fused with running denominator (tensor_tensor_reduce)
    PE transpose w row->cols, PE context matmuls (w stationary, enc natural)
At batch end: normalize by 1/denom, DMA attn row + context row out.
"""
import numpy as np

import concourse.bacc as bacc
import concourse.tile as tile
from concourse import mybir

N_CORES = 8
B, S, D, H = 32, 2048, 1024, 512
BPC = B // N_CORES          # batches per core = 4
SC = 512                    # s-chunk size
NCHUNK = S // SC            # 4
NU = SC // 128              # 128-row subtiles per chunk = 4
ND = D // 128               # d tiles = 8
NH = H // 128               # h tiles = 4

F32 = mybir.dt.float32
BF16 = mybir.dt.bfloat16
I32 = mybir.dt.int32
AF = mybir.ActivationFunctionType
OP = mybir.AluOpType


def _build():
    nc = bacc.Bacc("TRN2", target_bir_lowering=False)

    enc = nc.dram_tensor("enc", [BPC, S, D], F32, kind="ExternalInput")
    q_d = nc.dram_tensor("q", [BPC, D], F32, kind="ExternalInput")
    mask_d = nc.dram_tensor("mask_in", [BPC, S], I32, kind="ExternalInput")
    w1_d = nc.dram_tensor("W1", [H, D], F32, kind="ExternalInput")
    w1b_d = nc.dram_tensor("W1b", [H], F32, kind="ExternalInput")
    w2_d = nc.dram_tensor("W2", [H, D], F32, kind="ExternalInput")
    w2b_d = nc.dram_tensor("W2b", [H], F32, kind="ExternalInput")
    vw_d = nc.dram_tensor("Vw", [1, H], F32, kind="ExternalInput")
    vb_d = nc.dram_tensor("Vb", [1], F32, kind="ExternalInput")
    ident_d = nc.dram_tensor("ident", [128, 128], F32, kind="ExternalInput")

    ctx_out = nc.dram_tensor("ctx_out", [BPC, D], F32, kind="ExternalOutput")
    attn_out = nc.dram_tensor("attn_out", [BPC, S], F32, kind="ExternalOutput")

    with tile.TileContext(nc) as tc:
        with tc.tile_pool(name="const", bufs=1) as const:
            ident = const.tile([128, 128], F32)
            nc.sync.dma_start(ident[:], ident_d[:])
            ident_bf = const.tile([128, 128], BF16)
            nc.vector.tensor_copy(ident_bf[:], ident[:])

            w1t = const.tile([128, ND, H], BF16)      # [d=128, t, h]
            w2t = const.tile([128, ND, H], F32)
            qt = const.tile([128, ND, BPC], F32)      # [d=128, t, b]
            qp = const.tile([128, NH, BPC], F32)      # [h=128, j, b]
            vcol = const.tile([128, NH], BF16)        # [h=128, j]
            vb = const.tile([1, 1], F32)
            nc.sync.dma_start(vb[:], vb_d[:])
            maskadd = const.tile([1, BPC * S], F32)

            # ---- setup: transpose W1 (bf16), W2 (f32), q, biases, V ----
            with tc.tile_pool(name="setup", bufs=1) as setup, \
                 tc.tile_pool(name="setup_ps", bufs=2, space="PSUM") as sps, \
                 tc.tile_pool(name="setup_ps1", bufs=1, space="PSUM") as sps1:
                w1nat = []
                for j in range(NH):
                    t_ = setup.tile([128, D], BF16, tag=f"w1nat{j}")
                    nc.gpsimd.dma_start(t_[:], w1_d[j * 128:(j + 1) * 128, :])
                    w1nat.append(t_)
                for t in range(ND):
                    ps = sps.tile([128, NH, 128], BF16, tag="ps_t")
                    for j in range(NH):
                        nc.tensor.transpose(
                            ps[:, j, :], w1nat[j][:, t * 128:(t + 1) * 128],
                            ident_bf[:])
                    nc.vector.tensor_copy(
                        w1t[:, t, :], ps[:].rearrange("p a b -> p (a b)"))

                w2nat = []
                for j in range(NH):
                    t_ = setup.tile([128, D], F32, tag=f"w2nat{j}")
                    nc.sync.dma_start(t_[:], w2_d[j * 128:(j + 1) * 128, :])
                    w2nat.append(t_)
                for t in range(ND):
                    ps = sps.tile([128, NH, 128], F32, tag="ps_t2")
                    for j in range(NH):
                        nc.tensor.transpose(
                            ps[:, j, :], w2nat[j][:, t * 128:(t + 1) * 128],
                            ident[:])
                    nc.vector.tensor_copy(
                        w2t[:, t, :], ps[:].rearrange("p a b -> p (a b)"))

                # q [BPC, D] -> qT [128, t, b]
                q_sb = setup.tile([BPC, D], F32)
                nc.sync.dma_start(q_sb[:], q_d[:])
                ps_q = sps1.tile([128, ND, BPC], F32, tag="ps_q")
                for t in range(ND):
                    nc.tensor.transpose(
                        ps_q[:, t, :], q_sb[0:BPC, t * 128:(t + 1) * 128],
                        ident[0:BPC, 0:BPC])
                nc.vector.tensor_copy(qt[:], ps_q[:])

                # bias sum (W1_b + W2_b) -> column [128, j]
                b1 = setup.tile([1, H], F32)
                nc.sync.dma_start(b1[:], w1b_d[:])
                b2 = setup.tile([1, H], F32)
                nc.sync.dma_start(b2[:], w2b_d[:])
                bsum = setup.tile([1, H], F32)
                nc.vector.tensor_add(bsum[:], b1[:], b2[:])
                ps_b = sps1.tile([128, NH], F32, tag="ps_b")
                for j in range(NH):
                    nc.tensor.transpose(
                        ps_b[:, j:j + 1], bsum[0:1, j * 128:(j + 1) * 128],
                        ident[0:1, 0:1])
                bcol = setup.tile([128, NH], F32)
                nc.vector.tensor_copy(bcol[:], ps_b[:])

                # V row -> column bf16 [128, j]
                vrow = setup.tile([1, H], F32)
                nc.sync.dma_start(vrow[:], vw_d[:])
                ps_v = sps1.tile([128, NH], F32, tag="ps_v")
                for j in range(NH):
                    nc.tensor.transpose(
                        ps_v[:, j:j + 1], vrow[0:1, j * 128:(j + 1) * 128],
                        ident[0:1, 0:1])
                nc.vector.tensor_copy(vcol[:], ps_v[:])

                # qp[h, j, b] = sum_d W2[h,d] q[b,d] + (W1_b + W2_b)[h]
                for j in range(NH):
                    ps_qp = sps1.tile([128, BPC], F32, tag="ps_qp")
                    for t in range(ND):
                        nc.tensor.matmul(
                            ps_qp[:], w2t[:, t, j * 128:(j + 1) * 128],
                            qt[:, t, :], start=(t == 0), stop=(t == ND - 1))
                    nc.vector.tensor_scalar(
                        qp[:, j, :], ps_qp[:], scalar1=bcol[:, j:j + 1],
                        scalar2=None, op0=OP.add)

                # mask int32 -> f32
                mask_i = setup.tile([1, BPC * S], I32)
                nc.sync.dma_start(mask_i[:], mask_d[:])
                maskf = setup.tile([1, BPC * S], F32)
                nc.vector.tensor_copy(maskf[:], mask_i[:])
                # (mask - 1) * 1e9: 0 where mask==1, -1e9 where mask==0
                nc.vector.tensor_scalar(
                    maskadd[:], maskf[:], scalar1=1e9,
                    scalar2=-1e9, op0=OP.mult, op1=OP.add)

            # ---- main loop ----
            with tc.tile_pool(name="nat_p", bufs=12) as nat_p, \
                 tc.tile_pool(name="enct_p", bufs=3) as enct_p, \
                 tc.tile_pool(name="tanh_p", bufs=8) as tanh_p, \
                 tc.tile_pool(name="row_p", bufs=2) as row_p, \
                 tc.tile_pool(name="small_p", bufs=6) as small_p, \
                 tc.tile_pool(name="ps_tp", bufs=2, space="PSUM") as ps_tp, \
                 tc.tile_pool(name="ps_proj", bufs=2, space="PSUM") as ps_proj, \
                 tc.tile_pool(name="ps_small", bufs=1, space="PSUM") as ps_small, \
                 tc.tile_pool(name="ps_score", bufs=1, space="PSUM") as ps_score, \
                 tc.tile_pool(name="ps_ctx", bufs=1, space="PSUM") as ps_ctx:
                for b in range(BPC):
                    ctx_ps = ps_ctx.tile([1, 2, 512], F32, tag="ctx")
                    w_row = row_p.tile([1, S], F32, tag="w_row")
                    den4 = row_p.tile([1, NCHUNK], F32, tag="den4")
                    for c in range(NCHUNK):
                        nat = []
                        for u in range(NU):
                            t_ = nat_p.tile([128, D], BF16, tag="nat")
                            s0 = c * SC + u * 128
                            nc.gpsimd.dma_start(t_[:], enc[b, s0:s0 + 128, :])
                            nat.append(t_)
                        enct = enct_p.tile([128, ND, SC], BF16, tag="enct")
                        for t in range(ND):
                            tp = ps_tp.tile([128, NU, 128], BF16, tag="tp")
                            for u in range(NU):
                                nc.tensor.transpose(
                                    tp[:, u, :],
                                    nat[u][:, t * 128:(t + 1) * 128],
                                    ident_bf[:])
                            nc.vector.tensor_copy(
                                enct[:, t, :],
                                tp[:].rearrange("p a b -> p (a b)"))
                        tanhs = []
                        for j in range(NH):
                            proj = ps_proj.tile([128, SC], F32, tag="proj")
                            for t in range(ND):
                                nc.tensor.matmul(
                                    proj[:], w1t[:, t, j * 128:(j + 1) * 128],
                                    enct[:, t, :],
                                    start=(t == 0), stop=(t == ND - 1))
                            th = tanh_p.tile([128, SC], BF16, tag="tanh")
                            nc.scalar.activation(
                                th[:], proj[:], AF.Tanh,
                                bias=qp[:, j, b:b + 1])
                            tanhs.append(th)
                        score = ps_score.tile([1, SC], F32, tag="score")
                        for j in range(NH):
                            nc.tensor.matmul(
                                score[:], vcol[:, j:j + 1], tanhs[j][:],
                                start=(j == 0), stop=(j == NH - 1))
                        srow = small_p.tile([1, SC], F32, tag="srow")
                        nc.vector.tensor_add(
                            srow[:], score[:],
                            maskadd[0:1, b * S + c * SC:b * S + (c + 1) * SC])
                        nc.scalar.activation(
                            w_row[0:1, c * SC:(c + 1) * SC], srow[:], AF.Exp,
                            bias=vb[0:1, 0:1],
                            accum_out=den4[0:1, c:c + 1])
                        wc_ps = ps_small.tile([128, NU], F32, tag="wcol")
                        for k in range(NU):
                            nc.tensor.transpose(
                                wc_ps[:, k:k + 1],
                                w_row[0:1, c * SC + k * 128:c * SC + (k + 1) * 128],
                                ident[0:1, 0:1])
                        wcol = small_p.tile([128, NU], BF16, tag="wcol_sb")
                        nc.vector.tensor_copy(wcol[:], wc_ps[:])
                        for k in range(NU):
                            for h2 in range(2):
                                nc.tensor.matmul(
                                    ctx_ps[:, h2, :], wcol[:, k:k + 1],
                                    nat[k][:, h2 * 512:(h2 + 1) * 512],
                                    start=(c == 0 and k == 0),
                                    stop=(c == NCHUNK - 1 and k == NU - 1))
                    den = small_p.tile([1, 1], F32, tag="den")
                    nc.vector.reduce_sum(den[:], den4[:], axis=mybir.AxisListType.X)
                    recip = small_p.tile([1, 1], F32, tag="recip")
                    nc.vector.reciprocal(recip[:], den[:])
                    ctx_sb = row_p.tile([1, D], F32, tag="ctx_sb")
                    nc.vector.tensor_scalar(
                        ctx_sb[:], ctx_ps[:].rearrange("p a b -> p (a b)"),
                        scalar1=recip[0:1, 0:1], scalar2=None, op0=OP.mult)
                    nc.sync.dma_start(ctx_out[b:b + 1, :], ctx_sb[:])
                    attn_row = row_p.tile([1, S], F32, tag="attn_row")
                    nc.vector.tensor_scalar(
                        attn_row[:], w_row[:], scalar1=recip[0:1, 0:1],
                        scalar2=None, op0=OP.mult)
                    nc.sync.dma_start(attn_out[b:b + 1, :], attn_row[:])
    nc.finalize()
    return nc


_NC_CACHE = None


def _get_nc():
    global _NC_CACHE
    if _NC_CACHE is None:
        _NC_CACHE = _build()
    return _NC_CACHE


def kernel(hidden, encoder_outputs, mask, W1_w, W1_b, W2_w, W2_b, V_w, V_b):
    from concourse.bass_utils import run_bass_kernel_spmd

    hidden = np.asarray(hidden, dtype=np.float32)
    encoder_outputs = np.ascontiguousarray(
        np.asarray(encoder_outputs, dtype=np.float32))
    mask = np.ascontiguousarray(np.asarray(mask, dtype=np.int32))
    W1_w = np.asarray(W1_w, dtype=np.float32)
    W1_b = np.asarray(W1_b, dtype=np.float32)
    W2_w = np.asarray(W2_w, dtype=np.float32)
    W2_b = np.asarray(W2_b, dtype=np.float32)
    V_w = np.asarray(V_w, dtype=np.float32)
    V_b = np.asarray(V_b, dtype=np.float32)

    q = np.ascontiguousarray(hidden[-1])        # [B, D]
    ident = np.eye(128, dtype=np.float32)

    nc = _get_nc()
    in_maps = []
    for c in range(N_CORES):
        sl = slice(c * BPC, (c + 1) * BPC)
        in_maps.append({
            "enc": encoder_outputs[sl],
            "q": q[sl],
            "mask_in": mask[sl],
            "W1": W1_w, "W1b": W1_b, "W2": W2_w, "W2b": W2_b,
            "Vw": V_w, "Vb": V_b, "ident": ident,
        })
    res = run_bass_kernel_spmd(nc, in_maps, core_ids=list(range(N_CORES)))
    ctx = np.concatenate(
        [res.results[c]["ctx_out"] for c in range(N_CORES)], axis=0)
    attn = np.concatenate(
        [res.results[c]["attn_out"] for c in range(N_CORES)], axis=0)
    return ctx, attn


# revision 21
# speedup vs baseline: 1.1197x; 1.1197x over previous
"""Bahdanau attention forward on 8 Trainium2 NeuronCores (Bass/Tile).

Problem (per reference):
    query = hidden[-1]                                   [B, D]
    proj  = enc @ W1.T + W1_b + query @ W2.T + W2_b      [B, S, H]
    score = tanh(proj) @ V_w[0] + V_b                    [B, S]
    score = where(mask == 0, -1e9, score)
    attn  = softmax(score, axis=1)                       [B, S]
    ctx   = attn @ enc                                   [B, D]
    returns (ctx, attn)

Sharding: data-parallel over batch. B=32 -> 4 batches per core; params
replicated. Each core streams its 4x[S=2048, D=1024] encoder slice from HBM
exactly once (memory roofline ~32MB/core).

Per-core pipeline, per 512-row s-chunk:
    SWDGE cast-load enc f32->bf16 (natural [s,d])
    PE transpose 128x128 blocks -> encT [d,s] bf16 (proj needs d on partitions)
    PE proj matmuls (W1T stationary)  -> PSUM [h=128x4, s=512] f32
    ACT tanh(proj + qp_b) -> bf16     (qp = W2@q + b1 + b2, per-partition bias)
    PE score matmuls (V stationary)   -> PSUM [1, 512] f32
    ACT exp(score + V_b)              (unnormalized softmax; exp and tanh share
                                       one ACT table set)
    DVE exp*mask fused with running denominator (tensor_tensor_reduce)
    PE transpose w row->cols, PE context matmuls (w stationary, enc natural)
At batch end: normalize by 1/denom, DMA attn row + context row out.
"""
import numpy as np

import concourse.bacc as bacc
import concourse.tile as tile
from concourse import mybir

N_CORES = 8
B, S, D, H = 32, 2048, 1024, 512
BPC = B // N_CORES          # batches per core = 4
SC = 512                    # s-chunk size
NCHUNK = S // SC            # 4
NU = SC // 128              # 128-row subtiles per chunk = 4
ND = D // 128               # d tiles = 8
NH = H // 128               # h tiles = 4

F32 = mybir.dt.float32
BF16 = mybir.dt.bfloat16
I32 = mybir.dt.int32
AF = mybir.ActivationFunctionType
OP = mybir.AluOpType


def _build(reps=1, loop_reps=0, variant='full'):
    nc = bacc.Bacc("TRN2", target_bir_lowering=False)

    enc = nc.dram_tensor("enc", [BPC, S, D], F32, kind="ExternalInput")
    # Host-prearranged layouts (see kernel()): transposed weights, query,
    # V/bias columns, additive mask. All tiny next to enc.
    w1t_d = nc.dram_tensor("w1t_in", [128, ND, H], F32, kind="ExternalInput")
    qp_d = nc.dram_tensor("qp_in", [128, NH, BPC], F32, kind="ExternalInput")
    vcol_d = nc.dram_tensor("vcol_in", [128, NH], F32, kind="ExternalInput")
    maskcol_d = nc.dram_tensor("maskcol_in", [128, BPC * S // 128], F32,
                               kind="ExternalInput")
    ones_d = nc.dram_tensor("ones_in", [128, 16], F32, kind="ExternalInput")
    ident_d = nc.dram_tensor("ident", [128, 128], F32, kind="ExternalInput")

    ctx_out = nc.dram_tensor("ctx_out", [BPC, D], F32, kind="ExternalOutput")
    attn_out = nc.dram_tensor("attn_out", [BPC, S], F32, kind="ExternalOutput")

    with tile.TileContext(nc) as tc:
        with tc.tile_pool(name="const", bufs=1) as const:
            ident = const.tile([128, 128], F32)
            nc.sync.dma_start(ident[:], ident_d[:])
            ident_bf = const.tile([128, 128], BF16)
            nc.vector.tensor_copy(ident_bf[:], ident[:])

            w1t = const.tile([128, ND, H], BF16)      # [d=128, t, h]
            qp = const.tile([128, NH, BPC], F32)      # [h=128, j, b]
            vcol = const.tile([128, NH], BF16)        # [h=128, j]
            maskcol = const.tile([128, BPC * S // 128], F32)
            ones_f = const.tile([128, 16], F32)
            nc.sync.dma_start(ones_f[:], ones_d[:])
            ones_bf = const.tile([128, 1], BF16)
            nc.vector.tensor_copy(ones_bf[:], ones_f[:, 0:1])

            # ---- setup ----
            # First chunk's enc tiles load before the (larger) param DMAs so
            # the PE's first transposes start as early as possible.
            nat0 = []
            for u in range(NU):
                t_ = const.tile([128, D], BF16, tag=f"nat0_{u}")
                nc.gpsimd.dma_start(t_[:], enc[0, u * 128:(u + 1) * 128, :])
                nat0.append(t_)

            nc.gpsimd.dma_start(w1t[:], w1t_d[:])      # f32 -> bf16 cast
            nc.sync.dma_start(qp[:], qp_d[:])
            nc.gpsimd.dma_start(vcol[:], vcol_d[:])    # f32 -> bf16 cast
            nc.sync.dma_start(maskcol[:], maskcol_d[:])

            # ---- main loop ----
            import contextlib
            loop_cm = (tc.For_i(0, loop_reps, 1,
                                hint_engines=(mybir.EngineType.PE,))
                       if loop_reps else contextlib.nullcontext())
            with loop_cm, \
                 tc.tile_pool(name="nat_p", bufs=12) as nat_p, \
                 tc.tile_pool(name="enct_p", bufs=3) as enct_p, \
                 tc.tile_pool(name="tanh_p", bufs=8) as tanh_p, \
                 tc.tile_pool(name="row_p", bufs=3) as row_p, \
                 tc.tile_pool(name="small_p", bufs=6) as small_p, \
                 tc.tile_pool(name="ps_tp", bufs=1, space="PSUM") as ps_tp, \
                 tc.tile_pool(name="ps_proj", bufs=2, space="PSUM") as ps_proj, \
                 tc.tile_pool(name="ps_small", bufs=1, space="PSUM") as ps_small, \
                 tc.tile_pool(name="ps_score", bufs=1, space="PSUM") as ps_score, \
                 tc.tile_pool(name="ps_ctx", bufs=1, space="PSUM") as ps_ctx:
                for b in [bb for _ in range(reps) for bb in range(BPC)]:
                    ctx_ps = ps_ctx.tile([1, 2, 512], F32, tag="ctx")
                    # w columns for the whole batch: [s%128, s//128]
                    w_all = row_p.tile([128, S // 128], F32, tag="w_all")
                    den_ps = ps_small.tile([1, NCHUNK, NU], F32, tag="den")

                    # Deferred by one chunk so PE's context/denominator
                    # matmuls never stall on the score->exp->cast chain.
                    def emit_ctx(c, nat, wcol):
                        for k in range(NU):
                            for h2 in range(2):
                                nc.tensor.matmul(
                                    ctx_ps[:, h2, :], wcol[:, k:k + 1],
                                    nat[k][:, h2 * 512:(h2 + 1) * 512],
                                    start=(c == 0 and k == 0),
                                    stop=(c == NCHUNK - 1 and k == NU - 1))
                        nc.tensor.matmul(
                            den_ps[0:1, c, :], ones_bf[:], wcol[:],
                            start=True, stop=True)

                    pending = None
                    for c in range(NCHUNK):
                        if b == 0 and c == 0:
                            nat = nat0
                        else:
                            nat = []
                            for u in range(NU):
                                t_ = nat_p.tile([128, D], BF16, tag="nat")
                                s0 = c * SC + u * 128
                                nc.gpsimd.dma_start(t_[:], enc[b, s0:s0 + 128, :])
                                nat.append(t_)
                        if variant == "loads":
                            continue
                        enct = enct_p.tile([128, ND, SC], BF16, tag="enct")
                        for t in range(ND):
                            tp = ps_tp.tile([128, NU, 128], BF16, tag="tp")
                            for u in range(NU):
                                nc.tensor.transpose(
                                    tp[:, u, :],
                                    nat[u][:, t * 128:(t + 1) * 128],
                                    ident_bf[:])
                            nc.vector.tensor_copy(
                                enct[:, t, :],
                                tp[:].rearrange("p a b -> p (a b)"))
                        if variant == "loads_tp":
                            continue
                        tanhs = []
                        for j in range(NH):
                            proj = ps_proj.tile([128, SC], F32, tag="proj")
                            for t in range(ND):
                                nc.tensor.matmul(
                                    proj[:], w1t[:, t, j * 128:(j + 1) * 128],
                                    enct[:, t, :],
                                    start=(t == 0), stop=(t == ND - 1))
                            th = tanh_p.tile([128, SC], BF16, tag="tanh")
                            nc.scalar.activation(
                                th[:], proj[:], AF.Tanh,
                                bias=qp[:, j, b:b + 1])
                            tanhs.append(th)
                        # score columns: [s=128, k] = sum_j tanh[:,k].T @ v_j
                        score = ps_score.tile([128, NU], F32, tag="score")
                        for k in range(NU):
                            for j in range(NH):
                                nc.tensor.matmul(
                                    score[:, k:k + 1],
                                    tanhs[j][:, k * 128:(k + 1) * 128],
                                    vcol[:, j:j + 1],
                                    start=(j == 0), stop=(j == NH - 1))
                        # + additive mask (V_b folded in), then exp
                        srow = small_p.tile([128, NU], F32, tag="srow")
                        mc0 = b * (S // 128) + c * NU
                        nc.vector.tensor_add(
                            srow[:], score[:], maskcol[:, mc0:mc0 + NU])
                        nc.scalar.activation(
                            w_all[:, c * NU:(c + 1) * NU], srow[:], AF.Exp)
                        wcol = small_p.tile([128, NU], BF16, tag="wcol_sb")
                        nc.vector.tensor_copy(
                            wcol[:], w_all[:, c * NU:(c + 1) * NU])
                        if variant == "noctx":
                            continue
                        if pending is not None:
                            emit_ctx(*pending)
                        pending = (c, nat, wcol)
                    if variant in ("loads", "loads_tp", "noctx"):
                        continue
                    emit_ctx(*pending)

                    # denominator, reciprocal, outputs
                    den = small_p.tile([1, 1], F32, tag="den_sb")
                    nc.vector.reduce_sum(
                        den[:], den_ps[:].rearrange("p a b -> p (a b)"),
                        axis=mybir.AxisListType.X)
                    recip = small_p.tile([1, 1], F32, tag="recip")
                    nc.vector.reciprocal(recip[:], den[:])
                    ctx_sb = row_p.tile([1, D], F32, tag="ctx_sb")
                    nc.vector.tensor_scalar(
                        ctx_sb[:], ctx_ps[:].rearrange("p a b -> p (a b)"),
                        scalar1=recip[0:1, 0:1], scalar2=None, op0=OP.mult)
                    nc.sync.dma_start(ctx_out[b:b + 1, :], ctx_sb[:])
                    # attn rows: one wide transpose [128,16] -> [16,128],
                    # recip broadcast to 16 partitions via a K=1 matmul into
                    # the same psum tile, then scale + store.
                    wt_ps = ps_small.tile([16, 132], F32, tag="wt")
                    nc.tensor.transpose(
                        wt_ps[:, 0:128], w_all[:], ident[:])
                    nc.tensor.matmul(
                        wt_ps[:, 128:129], ones_f[0:1, 0:16],
                        recip[0:1, 0:1], start=True, stop=True)
                    attn_sb = row_p.tile([16, 128], F32, tag="attn_sb")
                    nc.vector.tensor_scalar(
                        attn_sb[:], wt_ps[:, 0:128],
                        scalar1=wt_ps[:, 128:129], scalar2=None, op0=OP.mult)
                    nc.sync.dma_start(
                        attn_out[b].rearrange("(g f) -> g f", g=16),
                        attn_sb[:])
    nc.finalize()
    return nc


_NC_CACHE = None


def _get_nc():
    global _NC_CACHE
    if _NC_CACHE is None:
        _NC_CACHE = _build()
    return _NC_CACHE


def kernel(hidden, encoder_outputs, mask, W1_w, W1_b, W2_w, W2_b, V_w, V_b):
    from concourse.bass_utils import run_bass_kernel_spmd

    hidden = np.asarray(hidden, dtype=np.float32)
    encoder_outputs = np.ascontiguousarray(
        np.asarray(encoder_outputs, dtype=np.float32))
    mask = np.ascontiguousarray(np.asarray(mask, dtype=np.int32))
    W1_w = np.asarray(W1_w, dtype=np.float32)
    W1_b = np.asarray(W1_b, dtype=np.float32)
    W2_w = np.asarray(W2_w, dtype=np.float32)
    W2_b = np.asarray(W2_b, dtype=np.float32)
    V_w = np.asarray(V_w, dtype=np.float32)
    V_b = np.asarray(V_b, dtype=np.float32)

    q = np.ascontiguousarray(hidden[-1])        # [B, D]
    ident = np.eye(128, dtype=np.float32)

    # Pre-arranged parameter layouts (replicated across cores):
    #   w1t/w2t: W.T [D, H] -> [128, ND, H] with d = t*128 + p
    #   bcol/vcol: [H] -> [128, NH] columns with h = j*128 + p
    w1t = np.ascontiguousarray(
        W1_w.T.reshape(ND, 128, H).transpose(1, 0, 2))
    vcol = np.ascontiguousarray(V_w[0].reshape(NH, 128).T)
    # additive mask in column layout with V_b folded in:
    # maskcol[b][p, g] = (mask[b, 128g+p]-1)*1e9 + V_b
    maskadd = ((mask.astype(np.float32) - 1.0) * 1e9 + V_b[0]).reshape(
        B, S // 128, 128)
    ones = np.ones((128, 16), dtype=np.float32)
    # qp[b, h] = q @ W2.T + W1_b + W2_b (tiny; done in f32 on host)
    qp_full = q @ W2_w.T + (W1_b + W2_b)[None, :]      # [B, H]

    nc = _get_nc()
    in_maps = []
    for c in range(N_CORES):
        sl = slice(c * BPC, (c + 1) * BPC)
        qp_c = np.ascontiguousarray(
            qp_full[sl].T.reshape(NH, 128, BPC).transpose(1, 0, 2))
        mc = np.ascontiguousarray(
            maskadd[sl].transpose(2, 0, 1).reshape(128, BPC * (S // 128), order="F"))
        mc = np.ascontiguousarray(
            np.concatenate([maskadd[bb].T for bb in range(sl.start, sl.stop)], axis=1))
        in_maps.append({
            "enc": encoder_outputs[sl],
            "w1t_in": w1t, "qp_in": qp_c,
            "vcol_in": vcol, "maskcol_in": mc,
            "ones_in": ones, "ident": ident,
        })
    res = run_bass_kernel_spmd(nc, in_maps, core_ids=list(range(N_CORES)))
    ctx = np.concatenate(
        [res.results[c]["ctx_out"] for c in range(N_CORES)], axis=0)
    attn = np.concatenate(
        [res.results[c]["attn_out"] for c in range(N_CORES)], axis=0)
    return ctx, attn
